# revision 13
# baseline (speedup 1.0000x reference)
"""Trainium2 Bass kernel for a BAN (bilinear attention network) layer, v2.

Reference computation (per batch b, head h, hd=64, scale=hd**-0.5):
    vp = (v @ Wv + bv)  -> [V=1024, 512] split into heads [h, V, 64]
    qp = (q @ Wq + bq)  -> [Q=512, 512]  split into heads [h, Q, 64]
    logits = vp_h @ att_w_h @ qp_h^T * scale        [V, Q]
    w = softmax(logits, axis=-1)
    pooled_v = mean_v(w @ qp_h)          [64]
    pooled_q = mean_q(w^T @ vp_h)        [64]
    fused = concat per head [pooled_v, pooled_q] -> [1024]
    out = relu(fused @ Wo + bo)          [512]

Algebra (validated ~4e-3 end-to-end in fp8/bf16 mixed precision):
  * pooled_q = (1/Q) * colsum_v(vp_h)  (softmax rows sum to 1)
  * pooled_v = z @ (q @ Wq)_h + bq_h with z = (1/V) sum_v e[v,:]/s[v]
  * att_w and the softmax scale fold into Wq on the host (Wqw)

v2 engine plan:
  * vp/qpw projections and the logits matmuls run in fp8e4 DoubleRow mode
    (2 contraction k-tiles per matmul, 0.5 cyc/row).  The per-head
    [32-partition, 2-ktile] operand layout is produced by small
    SBUF->SBUF relayout DMAs (DMA engines are otherwise idle).
  * exp runs on the Scalar engine over two-bank [128, 2, 512] psum tiles
    with NO accumulator; e is bf16.
  * softmax denominators s are free-axis DVE reduces (bf16 in, f32 out).
  * z columns: per 4-head group one psum region [4, 512]; each head's
    matmuls use a zero-padded [128, 4] bf16 stationary (col h%4 = r/V) so
    all 32 matmuls accumulate into one bank; a single [4,512] copy +
    tiny [4,128] transposes produce zT.
Sharding: data-parallel over batch, 2 batches per core, no collectives.
"""

import numpy as np
import ml_dtypes

BF16 = ml_dtypes.bfloat16
F8 = ml_dtypes.float8_e4m3fn

B, V_NUM, Q_NUM = 16, 1024, 512
V_DIM, Q_DIM = 256, 128
HIDDEN, HEADS, HD = 512, 8, 64
SCALE = HD ** -0.5

N_CORES = 8
BPC = B // N_CORES          # batches per core
DC = V_DIM // 128           # d-chunks of v (2)
IB = HIDDEN // 128          # i-blocks of hidden (4)
QC = Q_NUM // 128           # q-chunks (4)
VB = V_NUM // 512           # v-blocks of 512 (2)
VCH = V_NUM // 128          # v-chunks of 128 (8)
CP = VCH // 2               # v-chunk pairs (4)
NB = HIDDEN // 128          # out feature blocks (4)
KC = (2 * HEADS * HD) // 128  # fused feature chunks of 128 (8)

# fp8 quantization scales (powers of two; descales fold into copies/exp)
SV_IN = 64.0      # v, q inputs
SW = 2048.0       # Wv, Wqw weights
SP8 = 16.0        # vp, qpw activations (logits operands)
CP_DESCALE = SP8 / (SV_IN * SW)   # psum -> fp8 copy scale (2^-13)
EXP_SCALE = 1.0 / (SP8 * SP8)     # logits psum -> exp input scale (2^-8)

# param column offsets (bytes per partition, fp8-typed param)
O_VT8 = 0
O_QT8 = O_VT8 + BPC * 2 * V_NUM               # 4096
O_QTB = O_QT8 + BPC * 2 * Q_NUM               # 6144
O_WV8 = O_QTB + BPC * Q_NUM * 2               # 8192
O_WQW8 = O_WV8 + DC * HIDDEN                  # 9216
O_WQB = O_WQW8 + 2 * HIDDEN                   # 10240
O_ID = O_WQB + HIDDEN * 2                     # 11264
O_BALL = O_ID + 16                            # 11280
E_COLS = O_BALL + 20 * 4                      # 11360

L_VTB = 0
L_WVB = L_VTB + BPC * DC * V_NUM * 2          # 8192
L_WOB = L_WVB + DC * HIDDEN * 2               # 10240
L_COLS = L_WOB + KC * HIDDEN * 2              # 18432

_CACHE = {}


def _build_nc():
    from contextlib import ExitStack

    import concourse.bass as bass
    import concourse.tile as tile
    from concourse import bacc, mybir
    from concourse.tile import add_dep_helper

    f32 = mybir.dt.float32
    bf16 = mybir.dt.bfloat16
    fp8 = mybir.dt.float8e4
    u8 = mybir.dt.uint8
    AF = mybir.ActivationFunctionType
    ALU = mybir.AluOpType
    AX = mybir.AxisListType
    PM = mybir.MatmulPerfMode

    nc = bacc.Bacc("TRN2", target_bir_lowering=False)

    pE_p = nc.declare_dram_parameter("pE", [128, E_COLS], u8, isOutput=False)
    pL_p = nc.declare_dram_parameter("pL", [128, L_COLS], u8, isOutput=False)
    outT_p = nc.declare_dram_parameter("outT", [HIDDEN, BPC], f32, isOutput=True)

    with tile.TileContext(nc) as tc, ExitStack() as ctx:
        const = ctx.enter_context(tc.tile_pool(name="const", bufs=1))
        work = ctx.enter_context(tc.tile_pool(name="work", bufs=1))
        epool = ctx.enter_context(tc.tile_pool(name="epool", bufs=34))
        spool = ctx.enter_context(tc.tile_pool(name="spool", bufs=10))
        ps_pair = ctx.enter_context(tc.tile_pool(name="ps_pair", bufs=2, space="PSUM"))
        ps_z = ctx.enter_context(tc.tile_pool(name="ps_z", bufs=2, space="PSUM"))
        ps_tr = ctx.enter_context(tc.tile_pool(name="ps_tr", bufs=1, space="PSUM"))
        ps_sm = ctx.enter_context(tc.tile_pool(name="ps_sm", bufs=1, space="PSUM"))

        class SlotGuard:
            """Explicit WAR edges for psum slot reuse: the first writer of
            allocation i+bufs must wait for all readers of allocation i."""

            def __init__(self, bufs):
                self.bufs = bufs
                self.hist = []

            def alloc(self):
                self.hist.append([None, []])
                return len(self.hist) - 1

            def writer(self, idx, mi):
                if self.hist[idx][0] is None:
                    self.hist[idx][0] = mi
                    prev = idx - self.bufs
                    if prev >= 0:
                        for r in self.hist[prev][1]:
                            add_dep_helper(mi.ins, r.ins, sync=True,
                                           reason="psum slot WAR guard")
                return mi

            def reader(self, idx, mi):
                self.hist[idx][1].append(mi)
                return mi

        g_pair = SlotGuard(2)
        g_z = SlotGuard(2)
        g_tr = SlotGuard(1)
        g_sm = SlotGuard(1)

        # ---- inputs: two DMAs (early: fp8 operands; late: bf16 walls) ----
        pE_sb = const.tile([128, E_COLS], u8, tag="pE")
        pL_sb = const.tile([128, L_COLS], u8, tag="pL")
        nc.sync.dma_start(pE_sb[:], pE_p[:])
        nc.sync.dma_start(pL_sb[:], pL_p[:])

        vt8 = pE_sb[:, O_VT8:O_QT8].bitcast(fp8).rearrange(
            "p (b j v) -> p b j v", b=BPC, j=2)
        qt8 = pE_sb[:, O_QT8:O_QTB].bitcast(fp8).rearrange(
            "p (b j v) -> p b j v", b=BPC, j=2)
        qtb = pE_sb[:, O_QTB:O_WV8].bitcast(bf16).rearrange(
            "p (b v) -> p b v", b=BPC)
        wv8 = pE_sb[:, O_WV8:O_WQW8].bitcast(fp8).rearrange("p (c h) -> p c h", c=DC)
        wqw8 = pE_sb[:, O_WQW8:O_WQB].bitcast(fp8).rearrange("p (c h) -> p c h", c=2)
        wqb = pE_sb[:, O_WQB:O_ID].bitcast(bf16)
        identb = pE_sb[0:8, O_ID:O_BALL].bitcast(bf16)
        ball = pE_sb[:, O_BALL:E_COLS].bitcast(f32)
        bvs_sb = ball[:, 0:IB]
        bqws_sb = ball[:, IB:2 * IB]
        fb_sb = ball[:, 2 * IB:2 * IB + HEADS]
        bo_sb = ball[:, 2 * IB + HEADS:]

        vtb = pL_sb[:, L_VTB:L_WVB].bitcast(bf16).rearrange(
            "p (b c v) -> p b c v", b=BPC, c=DC)
        wvb = pL_sb[:, L_WVB:L_WOB].bitcast(bf16).rearrange(
            "p (c h) -> p c h", c=DC)
        wob = pL_sb[:, L_WOB:L_COLS].bitcast(bf16).rearrange(
            "p (k h) -> p k h", k=KC)

        # ---- long-lived activations ----
        vp8s = work.tile([128, BPC, IB, V_NUM], fp8, tag="vp8s")
        qpw8s = work.tile([128, BPC, IB, Q_NUM], fp8, tag="qpw8s")
        # relayout targets: head h = 2*ib + h2 lives at partition base 32*h2
        # (bass matmul bases allow 0/32/64 only), slab ib, ktile j along free;
        # partition 32*h2 + p holds local feature 32*j + p of head h
        vp8L = work.tile([64, BPC, IB, 2, V_NUM], fp8, tag="vp8L")
        qpw8L = work.tile([64, BPC, IB, 2, Q_NUM], fp8, tag="qpw8L")
        qp_sb = work.tile([128, BPC, QC, HIDDEN], bf16, tag="qp")
        zstack_sb = work.tile([4, 2, BPC, Q_NUM], bf16, tag="zstack")
        zT_sb = work.tile([128, BPC, QC, HEADS], bf16, tag="zT")
        fusedT_sb = work.tile([128, KC, BPC], bf16, tag="fused")
        outT_sb = work.tile([128, NB, BPC], f32, tag="outT")
        cv_sb = work.tile([128, BPC, DC], f32, tag="cv")
        cvb_sb = work.tile([128, BPC, DC], bf16, tag="cvb")
        # zero-padded z stationaries, one 4-col slab PER HEAD: slab m of
        # group g has only col m nonzero (r/V of head 4g+m), so each z matmul
        # adds exactly one row of the shared [4,512] psum region
        rb4_sb = work.tile([128, BPC, 2, 4, VCH, 4], bf16, tag="rb4")

        s_store = {}
        e_store = {}

        # zero the z-stationary padding before any writer/reader touches it
        for _b in range(BPC):
            for _g in range(2):
                nc.vector.memset(rb4_sb[:, _b, _g, :, :, :], 0.0)

        # ---- prologue thunks for batch b ----
        def prologue_thunks(b):
            thunks = []
            th = thunks.append

            for ib in range(IB):
                def vp_group(ib=ib):
                    ps = ps_pair.tile([128, 2, 512], f32, tag="pair")
                    gi = g_pair.alloc()
                    for vb in range(VB):
                        g_pair.writer(gi, nc.tensor.matmul(
                            ps[:, vb, :],
                            lhsT=wv8[:, :, ib * 128:(ib + 1) * 128],
                            rhs=vt8[:, b, :, vb * 512:(vb + 1) * 512],
                            start=True, stop=True, perf_mode=PM.DoubleRow))
                    for vb in range(VB):
                        g_pair.reader(gi, nc.vector.tensor_scalar(
                            vp8s[:, b, ib, vb * 512:(vb + 1) * 512],
                            ps[:, vb, :], CP_DESCALE, bvs_sb[:, ib:ib + 1],
                            ALU.mult, ALU.add))
                th(vp_group)

                def qpw_qp_group(ib=ib):
                    ps = ps_pair.tile([128, 2, 512], f32, tag="pair")
                    gi = g_pair.alloc()
                    g_pair.writer(gi, nc.tensor.matmul(
                        ps[:, 0, :],
                        lhsT=wqw8[0:64, :, ib * 128:(ib + 1) * 128],
                        rhs=qt8[0:64, b, :, :],
                        start=True, stop=True, perf_mode=PM.DoubleRow))
                    g_pair.writer(gi, nc.tensor.matmul(
                        ps[:, 1, :],
                        lhsT=qtb[:, b, ib * 128:(ib + 1) * 128],
                        rhs=wqb[:],
                        start=True, stop=True))
                    g_pair.reader(gi, nc.vector.tensor_scalar(
                        qpw8s[:, b, ib, :],
                        ps[:, 0, :], CP_DESCALE, bqws_sb[:, ib:ib + 1],
                        ALU.mult, ALU.add))
                    g_pair.reader(gi, nc.scalar.copy(
                        qp_sb[:, b, ib, :], ps[:, 1, :]))
                th(qpw_qp_group)

                def relayout(ib=ib):
                    for h2 in range(2):
                        for j in range(2):
                            src = slice(64 * h2 + 32 * j, 64 * h2 + 32 * j + 32)
                            dst = slice(32 * h2, 32 * h2 + 32)
                            nc.sync.dma_start(
                                vp8L[dst, b, ib, j, :], vp8s[src, b, ib, :])
                            nc.sync.dma_start(
                                qpw8L[dst, b, ib, j, :], qpw8s[src, b, ib, :])
                th(relayout)

            def cv_group():
                for dc in range(DC):
                    nc.vector.tensor_reduce(
                        cv_sb[:, b, dc:dc + 1], vtb[:, b, dc, :],
                        axis=AX.X, op=ALU.add)
                nc.vector.tensor_copy(cvb_sb[:, b, :], cv_sb[:, b, :])
            th(cv_group)

            for ib in range(IB):
                for half in range(2):
                    def pq_group(ib=ib, half=half):
                        h = 2 * ib + half
                        psq = ps_sm.tile([128, 8], f32, tag="sm")
                        gi = g_sm.alloc()
                        for dc in range(DC):
                            g_sm.writer(gi, nc.tensor.matmul(
                                psq[64:128, 0:1],
                                lhsT=wvb[:, dc, ib * 128 + 64 * half:
                                         ib * 128 + 64 * half + 64],
                                rhs=cvb_sb[:, b, dc:dc + 1],
                                start=(dc == 0), stop=(dc == DC - 1)))
                        g_sm.reader(gi, nc.vector.tensor_scalar(
                            fusedT_sb[64:128, h, b:b + 1], psq[64:128, 0:1],
                            1.0 / Q_NUM, fb_sb[64:128, h:h + 1],
                            ALU.mult, ALU.add))
                    th(pq_group)
            return thunks

        # ---- z drain machinery ----
        zbank = {}          # (b, g4group) -> (psum tile, guard idx)
        zmm_queue = []      # pending z matmul closures

        def make_zmm(b, h, cp, jc, start, stop):
            g = h // 4

            def zmm():
                zb, gi = zbank[(b, g)]
                g_z.writer(gi, nc.tensor.matmul(
                    zb[0:4, :],
                    lhsT=rb4_sb[:, b, g, h % 4, 2 * cp + jc, :],
                    rhs=e_store[(b, h, cp)][:, jc, :],
                    start=start, stop=stop, skip_group_check=True))
            return zmm

        def queue_pair_z(b, t):
            """Queue the 16 z matmuls for head-pair t of batch b."""
            hA, hB = 2 * t, 2 * t + 1
            g = hA // 4
            first = (t % 2 == 0)
            last = (t % 2 == 1)
            if first:
                zb = ps_z.tile([128, 512], f32, tag="z")
                zbank[(b, g)] = (zb, g_z.alloc())
            for cp in range(CP):
                for jc in range(2):
                    for h in (hA, hB):
                        st = first and cp == 0 and jc == 0 and h == hA
                        sp = last and cp == CP - 1 and jc == 1 and h == hB
                        zmm_queue.append(make_zmm(b, h, cp, jc, st, sp))

        def zrow_copy(b, g):
            zb, gi = zbank[(b, g)]
            g_z.reader(gi, nc.vector.tensor_copy(
                zstack_sb[0:4, g, b, :], zb[0:4, :]))

        # ---- ztail thunks: transposes + pooled_v for batch b ----
        def ztail_thunks(b):
            thunks = []
            th = thunks.append

            for g4 in range(2):
                def tr_group(g4=g4):
                    for qc in range(QC):
                        pst = ps_tr.tile([128, 8], bf16, tag="tr")
                        gi = g_tr.alloc()
                        g_tr.writer(gi, nc.tensor.transpose(
                            pst[:, 0:4],
                            zstack_sb[0:4, g4, b, qc * 128:(qc + 1) * 128],
                            identb[0:4, 0:4]))
                        g_tr.reader(gi, nc.vector.tensor_copy(
                            zT_sb[:, b, qc, 4 * g4:4 * g4 + 4], pst[:, 0:4]))
                th(tr_group)
            for h in range(HEADS):
                def pv_group(h=h):
                    psv = ps_sm.tile([128, 8], f32, tag="sm")
                    gi = g_sm.alloc()
                    for qc in range(QC):
                        g_sm.writer(gi, nc.tensor.matmul(
                            psv[0:64, 0:1],
                            lhsT=qp_sb[:, b, qc, h * 64:(h + 1) * 64],
                            rhs=zT_sb[:, b, qc, h:h + 1],
                            start=(qc == 0), stop=(qc == QC - 1)))
                    g_sm.reader(gi, nc.vector.tensor_scalar(
                        fusedT_sb[0:64, h, b:b + 1], psv[0:64, 0:1],
                        1.0, fb_sb[0:64, h:h + 1],
                        ALU.mult, ALU.add))
                th(pv_group)
            return thunks

        # ---- main loop for batch b ----
        def emit_main(b, pre_work, carry=None, final=False):
            for t in range(HEADS // 2):
                hA, hB = 2 * t, 2 * t + 1
                for h in (hA, hB):
                    s_store[(b, h)] = spool.tile([128, VCH], f32, name="s_t", tag="s")
                for cp in range(CP):
                    for h in (hA, hB):
                        pair = ps_pair.tile([128, 2, 512], f32, tag="pair")
                        gi = g_pair.alloc()
                        h2, hb = h % 2, h // 2
                        for jc in range(2):
                            c = 2 * cp + jc
                            g_pair.writer(gi, nc.tensor.matmul(
                                pair[:, jc, :],
                                lhsT=vp8L[32 * h2:32 * h2 + 32, b, hb, :,
                                          c * 128:(c + 1) * 128],
                                rhs=qpw8L[32 * h2:32 * h2 + 32, b, hb, :, :],
                                start=True, stop=True,
                                perf_mode=PM.DoubleRow))
                        e_t = epool.tile([128, 2, 512], bf16, name="e_t", tag="e")
                        e_store[(b, h, cp)] = e_t
                        g_pair.reader(gi, nc.scalar.activation(
                            e_t[:], pair[:], AF.Exp, scale=EXP_SCALE))
                        for jc in range(2):
                            nc.vector.tensor_reduce(
                                s_store[(b, h)][:, 2 * cp + jc:2 * cp + jc + 1],
                                e_t[:, jc, :], axis=AX.X, op=ALU.add)
                    # drain queued z matmuls: 4 per chunk-pair slot
                    for _ in range(4):
                        if zmm_queue:
                            zmm_queue.pop(0)()
                    # pre_work draws wait out pair 0 when it must follow a
                    # carried zrow_copy (ztail readers vs. its emission order)
                    for _ in range(2):
                        if pre_work and (t > 0 or carry is None):
                            pre_work.pop(0)()
                # reciprocals -> zero-padded stationaries
                for h in (hA, hB):
                    r_t = spool.tile([128, VCH], f32, tag="r")
                    nc.vector.reciprocal(r_t[:], s_store[(b, h)][:])
                    nc.vector.tensor_scalar_mul(
                        rb4_sb[:, b, h // 4, h % 4, :, h % 4], r_t[:],
                        1.0 / V_NUM)
                queue_pair_z(b, t)
                if t == 0 and carry is not None:
                    # previous batch's pair-3 z drained during our pair 0
                    carry()
                if t == 2:
                    # group 0 (pairs 0,1) z matmuls fully drained during
                    # pairs 1,2 -> its [4,512] row copy can fire now
                    zrow_copy(b, 0)
            if final:
                while zmm_queue:
                    zmm_queue.pop(0)()
                zrow_copy(b, 1)
            while pre_work:
                pre_work.pop(0)()

        # ---- schedule ----
        pro0 = prologue_thunks(0)
        for fn in pro0[:3]:
            fn()
        emit_main(0, pro0[3:] + prologue_thunks(1))
        emit_main(1, ztail_thunks(0), carry=lambda: zrow_copy(0, 1),
                  final=True)
        for fn in ztail_thunks(1):
            fn()

        # ---- epilogue: out = relu(fused @ Wo + bo), computed transposed ----
        for nb in range(NB):
            pso = ps_sm.tile([128, 8], f32, tag="sm")
            gi = g_sm.alloc()
            for kc in range(KC):
                g_sm.writer(gi, nc.tensor.matmul(
                    pso[:, 0:BPC],
                    lhsT=wob[:, kc, nb * 128:(nb + 1) * 128],
                    rhs=fusedT_sb[:, kc, :],
                    start=(kc == 0), stop=(kc == KC - 1)))
            g_sm.reader(gi, nc.scalar.activation(
                outT_sb[:, nb, :], pso[:, 0:BPC], AF.Relu,
                bias=bo_sb[:, nb:nb + 1]))
        nc.sync.dma_start(
            outT_p[:].rearrange("(o p) b -> p o b", p=128), outT_sb[:])

    nc.compile()
    return nc


def _get_nc():
    if "nc" not in _CACHE:
        _CACHE["nc"] = _build_nc()
    return _CACHE["nc"]


def _to8(x, s):
    return np.clip(x * s, -240.0, 240.0).astype(F8)


def _host_prep(v, q, Wv, bv, Wq, bq, att_w, Wo, bo):
    """Host-side layout transforms + weight folding. Returns per-core in_maps."""
    v = np.asarray(v, np.float32)
    q = np.asarray(q, np.float32)
    Wv = np.asarray(Wv, np.float32)
    bv = np.asarray(bv, np.float32)
    Wq = np.asarray(Wq, np.float32)
    bq = np.asarray(bq, np.float32)
    att_w = np.asarray(att_w, np.float32)
    Wo = np.asarray(Wo, np.float32)
    bo = np.asarray(bo, np.float32)

    # fold att_w and softmax scale into the q projection
    Wq_h = Wq.reshape(Q_DIM, HEADS, HD)
    Wqw = (SCALE * np.einsum("dhj,hij->dhi", Wq_h, att_w)).reshape(Q_DIM, HIDDEN)
    bqw = (SCALE * np.einsum("hj,hij->hi", bq.reshape(HEADS, HD), att_w)).reshape(HIDDEN)

    def as8(x):
        return np.ascontiguousarray(x).view(np.uint8)

    # shared weight columns (identical per core)
    wv8 = _to8(Wv.reshape(DC, 128, HIDDEN).transpose(1, 0, 2), SW)  # [128,2,512]
    wqw8_h = _to8(Wqw.reshape(2, 64, HIDDEN).transpose(1, 0, 2), SW)  # [64,2,512]
    wqw8 = np.zeros((128, 2, HIDDEN), F8)
    wqw8[0:64] = wqw8_h
    wqb = Wq.astype(BF16)                                           # [128,512]
    ident = np.zeros((128, 8), BF16)
    ident[:8, :8] = np.eye(8)
    bvs = (bv.reshape(IB, 128).T * SP8).astype(np.float32)
    bqws = (bqw.reshape(IB, 128).T * SP8).astype(np.float32)
    fb = np.concatenate(
        [bq.reshape(HEADS, HD).T,
         (V_NUM / Q_NUM) * bv.reshape(HEADS, HD).T], axis=0).astype(np.float32)
    boT = bo.reshape(NB, 128).T.astype(np.float32)
    ball = np.concatenate([bvs, bqws, fb, boT], axis=1)             # [128,20] f32

    shared_e = np.concatenate([
        as8(wv8.reshape(128, -1)), as8(wqw8.reshape(128, -1)),
        as8(wqb), as8(ident), as8(np.ascontiguousarray(ball))], axis=1)

    wvb = Wv.reshape(DC, 128, HIDDEN).transpose(1, 0, 2).astype(BF16)
    wob = Wo.reshape(KC, 128, HIDDEN).transpose(1, 0, 2).astype(BF16)
    shared_l = np.concatenate(
        [as8(wvb.reshape(128, -1)), as8(wob.reshape(128, -1))], axis=1)

    in_maps = []
    for i in range(N_CORES):
        sl = slice(i * BPC, (i + 1) * BPC)
        vsh, qsh = v[sl], q[sl]
        # vt8 [128, b, j, 1024]: [p, b, j, v] = v[b, v, 128j+p] * SV
        vt8 = _to8(vsh.transpose(2, 0, 1).reshape(2, 128, BPC, V_NUM)
                   .transpose(1, 2, 0, 3), SV_IN)
        # qt8 [64->128, b, j, 512]
        qt8_h = _to8(qsh.transpose(2, 0, 1).reshape(2, 64, BPC, Q_NUM)
                     .transpose(1, 2, 0, 3), SV_IN)
        qt8 = np.zeros((128, BPC, 2, Q_NUM), F8)
        qt8[0:64] = qt8_h
        # qtb bf16 [128, b, 512]
        qtb = qsh.transpose(2, 0, 1).astype(BF16)
        # vtb bf16 [128, b, dc, 1024]
        vtb = (vsh.transpose(2, 0, 1).reshape(DC, 128, BPC, V_NUM)
               .transpose(1, 2, 0, 3)).astype(BF16)
        pE = np.concatenate([
            as8(vt8.reshape(128, -1)), as8(qt8.reshape(128, -1)),
            as8(np.ascontiguousarray(qtb.reshape(128, -1))), shared_e], axis=1)
        pL = np.concatenate(
            [as8(vtb.reshape(128, -1)), shared_l], axis=1)
        in_maps.append({"pE": np.ascontiguousarray(pE),
                        "pL": np.ascontiguousarray(pL)})
    return in_maps


def kernel(**inputs):
    from concourse.bass_utils import run_bass_kernel_spmd

    nc = _get_nc()
    in_maps = _host_prep(**inputs)
    res = run_bass_kernel_spmd(nc, in_maps, core_ids=list(range(N_CORES)))
    out = np.empty((B, HIDDEN), np.float32)
    for i in range(N_CORES):
        out[i * BPC:(i + 1) * BPC] = np.asarray(res.results[i]["outT"]).T
    return out


# revision 15
# speedup vs baseline: 1.0531x; 1.0531x over previous
"""Trainium2 Bass kernel for a BAN (bilinear attention network) layer, v2.1.

Reference computation (per batch b, head h, hd=64, scale=hd**-0.5):
    vp = (v @ Wv + bv)  -> [V=1024, 512] split into heads [h, V, 64]
    qp = (q @ Wq + bq)  -> [Q=512, 512]  split into heads [h, Q, 64]
    logits = vp_h @ att_w_h @ qp_h^T * scale        [V, Q]
    w = softmax(logits, axis=-1)
    pooled_v = mean_v(w @ qp_h)          [64]
    pooled_q = mean_q(w^T @ vp_h)        [64]
    fused = concat per head [pooled_v, pooled_q] -> [1024]
    out = relu(fused @ Wo + bo)          [512]

Algebra:
  * pooled_q = (1/Q) * colsum_v(vp_h)  (softmax rows sum to 1)
  * pooled_v = z @ (q @ Wq)_h + bq_h with z = (1/V) sum_v e[v,:]/s[v]
  * att_w and the softmax scale fold into Wq on the host (Wqw)

Measured-physics engine plan (matmul cost ~= streamed output columns,
independent of dtype; fp8 DoubleRow = 2 contraction k-tiles per stream):
  * vp/qpw projections: fp8 DoubleRow, host-quantized inputs/weights
    (contraction 256/128 in one 512-col stream each).
  * logits: bf16 64-contract (the irreducible 65536 columns).
  * exp: Scalar engine over two-bank [128, 2, 512] psum pairs -> fp8 e.
    Heads with h%4==0 use two one-bank exps with accum_out (s on ActE);
    other heads get a single per-pair DVE reduce (s on DVE).
  * z: fp8 DoubleRow over chunk pairs (32768 columns), 4 heads of a group
    share one [4,512] psum region via zero-padded [128, 2, 4] fp8
    stationaries scaled by 2^18 (descaled in the pooled_v reader).
  * z matmuls drain two head-pairs late so their stationaries are always
    ready (no in-order PE stalls on the DVE reciprocal chain).
Sharding: data-parallel over batch, 2 batches per core, no collectives.
"""

import numpy as np
import ml_dtypes

BF16 = ml_dtypes.bfloat16
F8 = ml_dtypes.float8_e4m3fn

B, V_NUM, Q_NUM = 16, 1024, 512
V_DIM, Q_DIM = 256, 128
HIDDEN, HEADS, HD = 512, 8, 64
SCALE = HD ** -0.5

N_CORES = 8
BPC = B // N_CORES          # batches per core
DC = V_DIM // 128           # d-chunks of v (2)
IB = HIDDEN // 128          # i-blocks of hidden (4)
QC = Q_NUM // 128           # q-chunks (4)
VB = V_NUM // 512           # v-blocks of 512 (2)
VCH = V_NUM // 128          # v-chunks of 128 (8)
CP = VCH // 2               # v-chunk pairs (4)
NB = HIDDEN // 128          # out feature blocks (4)
KC = (2 * HEADS * HD) // 128  # fused feature chunks of 128 (8)

# fp8 quantization scales for the projections (powers of two)
SV_IN = 64.0      # v, q inputs
SW = 2048.0       # Wv, Wqw weights
CP_DESCALE = 1.0 / (SV_IN * SW)   # psum -> bf16 copy scale (2^-17)
ZSC = float(2.0 ** 18)            # rb4 stationary scale (fp8 range)

# param column offsets (bytes per partition, uint8-typed params)
O_VT8 = 0
O_QT8 = O_VT8 + BPC * 2 * V_NUM               # 4096
O_QTB = O_QT8 + BPC * 2 * Q_NUM               # 6144
O_WV8 = O_QTB + BPC * Q_NUM * 2               # 8192
O_WQW8 = O_WV8 + DC * HIDDEN                  # 9216
O_WQB = O_WQW8 + 2 * HIDDEN                   # 10240
O_ID = O_WQB + HIDDEN * 2                     # 11264
O_BALL = O_ID + 16                            # 11280
E_COLS = O_BALL + 20 * 4                      # 11360

L_VTB = 0
L_WVB = L_VTB + BPC * DC * V_NUM * 2          # 8192
L_WOB = L_WVB + DC * HIDDEN * 2               # 10240
L_COLS = L_WOB + KC * HIDDEN * 2              # 18432

_CACHE = {}


def _act_s_head(h):
    """Heads whose softmax denominator comes from the ActE accumulator."""
    return h % 4 == 0


def _build_nc():
    from contextlib import ExitStack

    import concourse.bass as bass
    import concourse.tile as tile
    from concourse import bacc, mybir
    from concourse.tile import add_dep_helper

    f32 = mybir.dt.float32
    bf16 = mybir.dt.bfloat16
    fp8 = mybir.dt.float8e4
    u8 = mybir.dt.uint8
    AF = mybir.ActivationFunctionType
    ALU = mybir.AluOpType
    AX = mybir.AxisListType
    PM = mybir.MatmulPerfMode

    nc = bacc.Bacc("TRN2", target_bir_lowering=False)

    pE_p = nc.declare_dram_parameter("pE", [128, E_COLS], u8, isOutput=False)
    pL_p = nc.declare_dram_parameter("pL", [128, L_COLS], u8, isOutput=False)
    outT_p = nc.declare_dram_parameter("outT", [HIDDEN, BPC], f32, isOutput=True)

    with tile.TileContext(nc) as tc, ExitStack() as ctx:
        const = ctx.enter_context(tc.tile_pool(name="const", bufs=1))
        work = ctx.enter_context(tc.tile_pool(name="work", bufs=1))
        epool = ctx.enter_context(tc.tile_pool(name="epool", bufs=34))
        spool = ctx.enter_context(tc.tile_pool(name="spool", bufs=10))
        ps_pair = ctx.enter_context(tc.tile_pool(name="ps_pair", bufs=2, space="PSUM"))
        ps_z = ctx.enter_context(tc.tile_pool(name="ps_z", bufs=2, space="PSUM"))
        ps_tr = ctx.enter_context(tc.tile_pool(name="ps_tr", bufs=1, space="PSUM"))
        ps_sm = ctx.enter_context(tc.tile_pool(name="ps_sm", bufs=1, space="PSUM"))

        class SlotGuard:
            """Explicit WAR edges for psum slot reuse: the first writer of
            allocation i+bufs must wait for all readers of allocation i."""

            def __init__(self, bufs):
                self.bufs = bufs
                self.hist = []

            def alloc(self):
                self.hist.append([None, []])
                return len(self.hist) - 1

            def writer(self, idx, mi):
                if self.hist[idx][0] is None:
                    self.hist[idx][0] = mi
                    prev = idx - self.bufs
                    if prev >= 0:
                        for r in self.hist[prev][1]:
                            add_dep_helper(mi.ins, r.ins, sync=True,
                                           reason="psum slot WAR guard")
                return mi

            def reader(self, idx, mi):
                self.hist[idx][1].append(mi)
                return mi

        g_pair = SlotGuard(2)
        g_z = SlotGuard(2)
        g_tr = SlotGuard(1)
        g_sm = SlotGuard(1)

        # ---- inputs: two DMAs (early: operands; late: bf16 walls) ----
        pE_sb = const.tile([128, E_COLS], u8, tag="pE")
        pL_sb = const.tile([128, L_COLS], u8, tag="pL")
        nc.sync.dma_start(pE_sb[:], pE_p[:])
        nc.sync.dma_start(pL_sb[:], pL_p[:])

        vt8 = pE_sb[:, O_VT8:O_QT8].bitcast(fp8).rearrange(
            "p (b j v) -> p b j v", b=BPC, j=2)
        qt8 = pE_sb[:, O_QT8:O_QTB].bitcast(fp8).rearrange(
            "p (b j v) -> p b j v", b=BPC, j=2)
        qtb = pE_sb[:, O_QTB:O_WV8].bitcast(bf16).rearrange(
            "p (b v) -> p b v", b=BPC)
        wv8 = pE_sb[:, O_WV8:O_WQW8].bitcast(fp8).rearrange(
            "p (c h) -> p c h", c=DC)
        wqw8 = pE_sb[:, O_WQW8:O_WQB].bitcast(fp8).rearrange(
            "p (c h) -> p c h", c=2)
        wqb = pE_sb[:, O_WQB:O_ID].bitcast(bf16)
        identb = pE_sb[0:8, O_ID:O_BALL].bitcast(bf16)
        ball = pE_sb[:, O_BALL:E_COLS].bitcast(f32)
        bvs_sb = ball[:, 0:IB]
        bqws_sb = ball[:, IB:2 * IB]
        fb_sb = ball[:, 2 * IB:2 * IB + HEADS]
        bo_sb = ball[:, 2 * IB + HEADS:]

        vtb = pL_sb[:, L_VTB:L_WVB].bitcast(bf16).rearrange(
            "p (b c v) -> p b c v", b=BPC, c=DC)
        wvb = pL_sb[:, L_WVB:L_WOB].bitcast(bf16).rearrange(
            "p (c h) -> p c h", c=DC)
        wob = pL_sb[:, L_WOB:L_COLS].bitcast(bf16).rearrange(
            "p (k h) -> p k h", k=KC)

        # ---- long-lived activations ----
        vpT_sb = work.tile([128, BPC, IB, V_NUM], bf16, tag="vpT")
        qpwT_sb = work.tile([128, BPC, IB, Q_NUM], bf16, tag="qpwT")
        qp_sb = work.tile([128, BPC, QC, HIDDEN], bf16, tag="qp")
        zstack_sb = work.tile([4, 2, BPC, Q_NUM], bf16, tag="zstack")
        zT_sb = work.tile([128, BPC, QC, HEADS], bf16, tag="zT")
        fusedT_sb = work.tile([128, KC, BPC], bf16, tag="fused")
        outT_sb = work.tile([128, NB, BPC], f32, tag="outT")
        cv_sb = work.tile([128, BPC, DC], f32, tag="cv")
        cvb_sb = work.tile([128, BPC, DC], bf16, tag="cvb")
        # zero-padded z stationaries, one [cp, jc, 16]-slab PER HEAD: slab m
        # of group g has only col m nonzero (r * 2^18 / V of head 4g+m), so
        # each DoubleRow z matmul adds exactly one row of the shared psum
        # region (16-wide cols: dual-fp8 ldweights needs 16B-aligned steps)
        rb4_sb = work.tile([128, BPC, 2, 4, CP, 2, 16], fp8, tag="rb4")

        s_store = {}
        e_store = {}

        # zero the z-stationary padding before any writer/reader touches it
        for _b in range(BPC):
            for _g in range(2):
                nc.vector.memset(rb4_sb[:, _b, _g, :, :, :, :], 0.0)

        # ---- prologue thunks for batch b ----
        def prologue_thunks(b):
            thunks = []
            th = thunks.append

            for ib in range(IB):
                def vp_group(ib=ib):
                    ps = ps_pair.tile([128, 2, 512], f32, tag="pair")
                    gi = g_pair.alloc()
                    for vb in range(VB):
                        g_pair.writer(gi, nc.tensor.matmul(
                            ps[:, vb, :],
                            lhsT=wv8[:, :, ib * 128:(ib + 1) * 128],
                            rhs=vt8[:, b, :, vb * 512:(vb + 1) * 512],
                            start=True, stop=True, perf_mode=PM.DoubleRow))
                    for vb in range(VB):
                        g_pair.reader(gi, nc.vector.tensor_scalar(
                            vpT_sb[:, b, ib, vb * 512:(vb + 1) * 512],
                            ps[:, vb, :], CP_DESCALE, bvs_sb[:, ib:ib + 1],
                            ALU.mult, ALU.add))
                th(vp_group)

                def qpw_qp_group(ib=ib):
                    ps = ps_pair.tile([128, 2, 512], f32, tag="pair")
                    gi = g_pair.alloc()
                    g_pair.writer(gi, nc.tensor.matmul(
                        ps[:, 0, :],
                        lhsT=wqw8[0:64, :, ib * 128:(ib + 1) * 128],
                        rhs=qt8[0:64, b, :, :],
                        start=True, stop=True, perf_mode=PM.DoubleRow))
                    g_pair.writer(gi, nc.tensor.matmul(
                        ps[:, 1, :],
                        lhsT=qtb[:, b, ib * 128:(ib + 1) * 128],
                        rhs=wqb[:],
                        start=True, stop=True))
                    g_pair.reader(gi, nc.vector.tensor_scalar(
                        qpwT_sb[:, b, ib, :],
                        ps[:, 0, :], CP_DESCALE, bqws_sb[:, ib:ib + 1],
                        ALU.mult, ALU.add))
                    g_pair.reader(gi, nc.scalar.copy(
                        qp_sb[:, b, ib, :], ps[:, 1, :]))
                th(qpw_qp_group)

            def cv_group():
                for dc in range(DC):
                    nc.vector.tensor_reduce(
                        cv_sb[:, b, dc:dc + 1], vtb[:, b, dc, :],
                        axis=AX.X, op=ALU.add)
                nc.vector.tensor_copy(cvb_sb[:, b, :], cv_sb[:, b, :])
            th(cv_group)

            for ib in range(IB):
                for half in range(2):
                    def pq_group(ib=ib, half=half):
                        h = 2 * ib + half
                        psq = ps_sm.tile([128, 8], f32, tag="sm")
                        gi = g_sm.alloc()
                        for dc in range(DC):
                            g_sm.writer(gi, nc.tensor.matmul(
                                psq[64:128, 0:1],
                                lhsT=wvb[:, dc, ib * 128 + 64 * half:
                                         ib * 128 + 64 * half + 64],
                                rhs=cvb_sb[:, b, dc:dc + 1],
                                start=(dc == 0), stop=(dc == DC - 1)))
                        g_sm.reader(gi, nc.vector.tensor_scalar(
                            fusedT_sb[64:128, h, b:b + 1], psq[64:128, 0:1],
                            1.0 / Q_NUM, fb_sb[64:128, h:h + 1],
                            ALU.mult, ALU.add))
                    th(pq_group)
            return thunks

        # ---- z drain machinery (fp8 DoubleRow over chunk pairs) ----
        zbank = {}          # (b, group) -> (psum tile, guard idx)
        zmm_queue = []

        def make_zmm(b, h, cp, start, stop):
            g = h // 4

            def zmm():
                zb, gi = zbank[(b, g)]
                g_z.writer(gi, nc.tensor.matmul(
                    zb[0:16, :],
                    lhsT=rb4_sb[:, b, g, h % 4, cp, :, :],
                    rhs=e_store[(b, h, cp)][:],
                    start=start, stop=stop, perf_mode=PM.DoubleRow,
                    skip_group_check=True))
            return zmm

        def queue_pair_z(b, t):
            """Queue the 8 DoubleRow z matmuls for head-pair t of batch b."""
            hA, hB = 2 * t, 2 * t + 1
            g = hA // 4
            first = (t % 2 == 0)
            last = (t % 2 == 1)
            if first:
                zb = ps_z.tile([128, 512], f32, tag="z")
                zbank[(b, g)] = (zb, g_z.alloc())
            for cp in range(CP):
                for h in (hA, hB):
                    st = first and cp == 0 and h == hA
                    sp = last and cp == CP - 1 and h == hB
                    zmm_queue.append(make_zmm(b, h, cp, st, sp))

        def zrow_copy(b, g):
            zb, gi = zbank[(b, g)]
            g_z.reader(gi, nc.vector.tensor_copy(
                zstack_sb[0:4, g, b, :], zb[0:4, :]))

        # ---- ztail thunks for (batch, head-group): transposes + pooled_v ----
        def ztail_thunks(b, g4):
            thunks = []
            th = thunks.append

            def tr_group(g4=g4):
                for qc in range(QC):
                    pst = ps_tr.tile([128, 8], bf16, tag="tr")
                    gi = g_tr.alloc()
                    g_tr.writer(gi, nc.tensor.transpose(
                        pst[:, 0:4],
                        zstack_sb[0:4, g4, b, qc * 128:(qc + 1) * 128],
                        identb[0:4, 0:4]))
                    g_tr.reader(gi, nc.vector.tensor_copy(
                        zT_sb[:, b, qc, 4 * g4:4 * g4 + 4], pst[:, 0:4]))
            th(tr_group)
            for h in range(4 * g4, 4 * g4 + 4):
                def pv_group(h=h):
                    psv = ps_sm.tile([128, 8], f32, tag="sm")
                    gi = g_sm.alloc()
                    for qc in range(QC):
                        g_sm.writer(gi, nc.tensor.matmul(
                            psv[0:64, 0:1],
                            lhsT=qp_sb[:, b, qc, h * 64:(h + 1) * 64],
                            rhs=zT_sb[:, b, qc, h:h + 1],
                            start=(qc == 0), stop=(qc == QC - 1)))
                    g_sm.reader(gi, nc.vector.tensor_scalar(
                        fusedT_sb[0:64, h, b:b + 1], psv[0:64, 0:1],
                        1.0 / ZSC, fb_sb[0:64, h:h + 1],
                        ALU.mult, ALU.add))
                th(pv_group)
            return thunks

        # ---- main loop for batch b ----
        def emit_main(b, pre_work, carry=None, final=False):
            draw_from = 2 if carry is not None else 0
            for t in range(HEADS // 2):
                hA, hB = 2 * t, 2 * t + 1
                for h in (hA, hB):
                    s_store[(b, h)] = spool.tile(
                        [128, VCH], f32, name="s_t", tag="s")
                for cp in range(CP):
                    for h in (hA, hB):
                        pair = ps_pair.tile([128, 2, 512], f32, tag="pair")
                        gi = g_pair.alloc()
                        h2, hb = h % 2, h // 2
                        for jc in range(2):
                            c = 2 * cp + jc
                            g_pair.writer(gi, nc.tensor.matmul(
                                pair[:, jc, :],
                                lhsT=vpT_sb[64 * h2:64 * h2 + 64, b, hb,
                                            c * 128:(c + 1) * 128],
                                rhs=qpwT_sb[64 * h2:64 * h2 + 64, b, hb, :],
                                start=True, stop=True))
                        e_t = epool.tile([128, 2, 512], fp8, name="e_t",
                                         tag="e")
                        e_store[(b, h, cp)] = e_t
                        if _act_s_head(h):
                            for jc in range(2):
                                c = 2 * cp + jc
                                g_pair.reader(gi, nc.scalar.activation(
                                    e_t[:, jc, :], pair[:, jc, :], AF.Exp,
                                    accum_out=s_store[(b, h)][:, c:c + 1]))
                        else:
                            g_pair.reader(gi, nc.scalar.activation(
                                e_t[:], pair[:], AF.Exp))
                            nc.vector.tensor_reduce(
                                s_store[(b, h)][:, 2 * cp:2 * cp + 2],
                                e_t[:], axis=AX.X, op=ALU.add)
                    # drain queued z matmuls (from head-pair t-2): 2 per slot
                    for _ in range(2):
                        if zmm_queue:
                            zmm_queue.pop(0)()
                    for _ in range(2):
                        if pre_work and t >= draw_from:
                            pre_work.pop(0)()
                # reciprocals -> zero-padded fp8 stationaries (x 2^18 / V)
                for h in (hA, hB):
                    r_t = spool.tile([128, VCH], f32, name="r_t", tag="r")
                    nc.vector.reciprocal(r_t[:], s_store[(b, h)][:])
                    nc.vector.tensor_scalar_mul(
                        rb4_sb[:, b, h // 4, h % 4, :, :, h % 4], r_t[:],
                        ZSC / V_NUM)
                queue_pair_z(b, t)
                if t == 1 and carry is not None:
                    # previous batch's group-1 z drained during our pairs 0,1
                    carry()
                if t == 2:
                    # group 0 (pairs 0,1) z drained during pairs 1 (tail), 2
                    zrow_copy(b, 0)
                    pre_work.extend(ztail_thunks(b, 0))
            if final:
                while zmm_queue:
                    zmm_queue.pop(0)()
                zrow_copy(b, 1)
            while pre_work:
                pre_work.pop(0)()

        # ---- schedule ----
        pro0 = prologue_thunks(0)
        for fn in pro0[:3]:
            fn()
        emit_main(0, pro0[3:] + prologue_thunks(1))

        def carry1():
            zrow_copy(0, 1)
            # late ztail for batch-0 group 1 rides the batch-1 stream
            main1_pre.extend(ztail_thunks(0, 1))

        main1_pre = []
        emit_main(1, main1_pre, carry=carry1, final=True)
        for fn in ztail_thunks(1, 1):
            fn()

        # ---- epilogue: out = relu(fused @ Wo + bo), computed transposed ----
        for nb in range(NB):
            pso = ps_sm.tile([128, 8], f32, tag="sm")
            gi = g_sm.alloc()
            for kc in range(KC):
                g_sm.writer(gi, nc.tensor.matmul(
                    pso[:, 0:BPC],
                    lhsT=wob[:, kc, nb * 128:(nb + 1) * 128],
                    rhs=fusedT_sb[:, kc, :],
                    start=(kc == 0), stop=(kc == KC - 1)))
            g_sm.reader(gi, nc.scalar.activation(
                outT_sb[:, nb, :], pso[:, 0:BPC], AF.Relu,
                bias=bo_sb[:, nb:nb + 1]))
        nc.sync.dma_start(
            outT_p[:].rearrange("(o p) b -> p o b", p=128), outT_sb[:])

    nc.compile()
    return nc


def _get_nc():
    if "nc" not in _CACHE:
        _CACHE["nc"] = _build_nc()
    return _CACHE["nc"]


def _to8(x, s):
    return np.clip(x * s, -240.0, 240.0).astype(F8)


def _host_prep(v, q, Wv, bv, Wq, bq, att_w, Wo, bo):
    """Host-side layout transforms + weight folding. Returns per-core in_maps."""
    v = np.asarray(v, np.float32)
    q = np.asarray(q, np.float32)
    Wv = np.asarray(Wv, np.float32)
    bv = np.asarray(bv, np.float32)
    Wq = np.asarray(Wq, np.float32)
    bq = np.asarray(bq, np.float32)
    att_w = np.asarray(att_w, np.float32)
    Wo = np.asarray(Wo, np.float32)
    bo = np.asarray(bo, np.float32)

    # fold att_w and softmax scale into the q projection
    Wq_h = Wq.reshape(Q_DIM, HEADS, HD)
    Wqw = (SCALE * np.einsum("dhj,hij->dhi", Wq_h, att_w)).reshape(Q_DIM, HIDDEN)
    bqw = (SCALE * np.einsum("hj,hij->hi", bq.reshape(HEADS, HD), att_w)).reshape(HIDDEN)

    def as8(x):
        return np.ascontiguousarray(x).view(np.uint8)

    # shared weight columns (identical per core)
    wv8 = _to8(Wv.reshape(DC, 128, HIDDEN).transpose(1, 0, 2), SW)  # [128,2,512]
    wqw8_h = _to8(Wqw.reshape(2, 64, HIDDEN).transpose(1, 0, 2), SW)  # [64,2,512]
    wqw8 = np.zeros((128, 2, HIDDEN), F8)
    wqw8[0:64] = wqw8_h
    wqb = Wq.astype(BF16)                                           # [128,512]
    ident = np.zeros((128, 8), BF16)
    ident[:8, :8] = np.eye(8)
    bvs = bv.reshape(IB, 128).T.astype(np.float32)
    bqws = bqw.reshape(IB, 128).T.astype(np.float32)
    fb = np.concatenate(
        [bq.reshape(HEADS, HD).T,
         (V_NUM / Q_NUM) * bv.reshape(HEADS, HD).T], axis=0).astype(np.float32)
    boT = bo.reshape(NB, 128).T.astype(np.float32)
    ball = np.concatenate([bvs, bqws, fb, boT], axis=1)             # [128,20] f32

    shared_e = np.concatenate([
        as8(wv8.reshape(128, -1)), as8(wqw8.reshape(128, -1)),
        as8(wqb), as8(ident), as8(np.ascontiguousarray(ball))], axis=1)

    wvb = Wv.reshape(DC, 128, HIDDEN).transpose(1, 0, 2).astype(BF16)
    wob = Wo.reshape(KC, 128, HIDDEN).transpose(1, 0, 2).astype(BF16)
    shared_l = np.concatenate(
        [as8(wvb.reshape(128, -1)), as8(wob.reshape(128, -1))], axis=1)

    in_maps = []
    for i in range(N_CORES):
        sl = slice(i * BPC, (i + 1) * BPC)
        vsh, qsh = v[sl], q[sl]
        # vt8 [128, b, j, 1024]: [p, b, j, v] = v[b, v, 128j+p] * SV
        vt8 = _to8(vsh.transpose(2, 0, 1).reshape(2, 128, BPC, V_NUM)
                   .transpose(1, 2, 0, 3), SV_IN)
        # qt8 [64->128, b, j, 512]
        qt8_h = _to8(qsh.transpose(2, 0, 1).reshape(2, 64, BPC, Q_NUM)
                     .transpose(1, 2, 0, 3), SV_IN)
        qt8 = np.zeros((128, BPC, 2, Q_NUM), F8)
        qt8[0:64] = qt8_h
        # qtb bf16 [128, b, 512]
        qtb = qsh.transpose(2, 0, 1).astype(BF16)
        # vtb bf16 [128, b, dc, 1024]
        vtb = (vsh.transpose(2, 0, 1).reshape(DC, 128, BPC, V_NUM)
               .transpose(1, 2, 0, 3)).astype(BF16)
        pE = np.concatenate([
            as8(vt8.reshape(128, -1)), as8(qt8.reshape(128, -1)),
            as8(np.ascontiguousarray(qtb.reshape(128, -1))), shared_e], axis=1)
        pL = np.concatenate(
            [as8(vtb.reshape(128, -1)), shared_l], axis=1)
        in_maps.append({"pE": np.ascontiguousarray(pE),
                        "pL": np.ascontiguousarray(pL)})
    return in_maps


def kernel(**inputs):
    from concourse.bass_utils import run_bass_kernel_spmd

    nc = _get_nc()
    in_maps = _host_prep(**inputs)
    res = run_bass_kernel_spmd(nc, in_maps, core_ids=list(range(N_CORES)))
    out = np.empty((B, HIDDEN), np.float32)
    for i in range(N_CORES):
        out[i * BPC:(i + 1) * BPC] = np.asarray(res.results[i]["outT"]).T
    return out
